# revision 74
# baseline (speedup 1.0000x reference)
"""Trainium2 Bass kernel for nn_CRFTModule (moe_routing).

Pure data parallel over batch: 8 cores, one batch row (4096 tokens) each.

Math per core (batch b, S=4096 tokens, H=1024):
  z      = gelu(x @ W1 + b1) @ W2 + b2              (critical-path detector)
  mask   = z > logit(0.7)                            (compare in logit space)
  aw     = softmax(x[last] @ sel_w + sel_b)          (adapter selector, 4-way)
  t      = gelu(x @ Dcat + db)                       (all 4 down-projs, [S,32])
  wm     = 0.3 * mask * (sum_a aw[a] (t_a @ up_w[a] + up_b[a]))
  out    = x + wm

Device pipeline (fp8e4 DoubleRow matmuls for the wide GEMMs, fp32 psum):
  - x loaded naturally [tok, H] as float32r (one 2 MiB DMA per tile, all
    loads queued ahead of the stores on the FIFO DMA device), transposed
    on PE (f32r: 1.5 cyc/row, 128x128 blocks); the PSUM->SBUF copies
    (6 ACT / 2 DVE per tile) cast to fp8e4 and pack k-tile PAIRS
    [128, 2, T] so mm1/down run DoubleRow (2 k-tiles of 128, 0.5 cyc/row).
  - detector mm1: fp8 DoubleRow; gelu -> f16 Hs; mm2 f16 back to NATURAL
    orientation so the mask is a per-partition [tok,1] scalar.
  - adapter path: down fp8 DoubleRow -> gelu -> G fp8 (single 36-row copy,
    ones rows pre-set once); up fp8 with the softmax weights folded into
    the fp8 up matrix; final op fuses mask-mult + residual-add on DVE,
    writing an f16 staging tile.
  - fp16 output (the 2e-2 rel-err budget dwarfs fp16 rounding ~5e-4):
    halves HBM write traffic; plain HWDGE f16 stores, host upcasts.
  - software pipeline: block i runs [mm1(i), down(i), up+stt(i-1),
    transposes(i+1), mm2(i)] so the DVE STT chain starts early on inputs
    a full block old; last 2 pre-epilogue tiles run their own up+stt
    (lean tail), endgame copies all ride ACT to keep DVE clear.
  - softmax via the tanh identity (one ACT table set).

Measured (TimelineSim cost model): 82.1 us vs 127.3 us baseline; device
rel err 6.2e-3 (fp8 mask flips ~0.2% of tokens, fp16 store ~5e-4).
"""
import math

import numpy as np
import ml_dtypes

import concourse.bacc as bacc
import concourse.mybir as mybir
from concourse.bass import ts
from concourse.tile import TileContext
from concourse.bass_utils import run_bass_kernel_spmd

dt = mybir.dt
AF = mybir.ActivationFunctionType
ALU = mybir.AluOpType
DR = mybir.MatmulPerfMode.DoubleRow

B, S, H = 8, 4096, 1024
A_DIM, N_ADAPT = 8, 4
PD = H // 2              # 512 detector hidden dim
SCALE = 0.3
THRESH = 0.7
N_CORES = 8
T = 512                  # tokens per tile
N_TILES = S // T         # 8
KUP = N_ADAPT * A_DIM + N_ADAPT  # 36

XBUFS = 8

# fp8 const blob columns: w1 (8*512) | dcat (8*32)
_F8_COLS = 8 * PD + 8 * 32          # 4352
# f16 const blob columns: w2 (4)
_F16_COLS = 4
# f32 const blob columns: b1(4) thr(1) selw(32) db(1) selb(1) o14(4) i4(4)
#                         e36(36)
_F32_COLS = 83


def _build():
    nc = bacc.Bacc("TRN2", target_bir_lowering=False, debug=False)

    x = nc.declare_dram_parameter("x", [S, H], dt.float32r, isOutput=False)
    fb8 = nc.declare_dram_parameter("fb8", [128, _F8_COLS], dt.float8e4, isOutput=False)
    fb16 = nc.declare_dram_parameter("fb16", [128, _F16_COLS], dt.float16, isOutput=False)
    u36p = nc.declare_dram_parameter("u36p", [KUP, H], dt.float16, isOutput=False)
    fb32 = nc.declare_dram_parameter("fb32", [128, _F32_COLS], dt.float32, isOutput=False)
    idp = nc.declare_dram_parameter("idp", [128, 128], dt.float32r, isOutput=False)
    # fp16 output: the 2e-2 rel-err budget dwarfs fp16 rounding (~5e-4); halves
    # HBM write traffic. Host upcasts to fp32.
    out = nc.declare_dram_parameter("out", [S, H], dt.float16, isOutput=True)

    with TileContext(nc) as tc:
        with (
            tc.tile_pool(name="consts", bufs=1) as cp,
            tc.tile_pool(name="work", bufs=2) as wp,
            tc.tile_pool(name="psum", bufs=2, space="PSUM") as pp,
        ):
            # ---- tile-0 activations first, split along H so chunk pair q
            # is transposable right after its own quarter lands ----
            X = [None] * N_TILES
            X[0] = wp.tile([128, 4, H], dt.float32r, tag="X", name="X0", bufs=XBUFS)
            nc.sync.dma_start(
                out=X[0][:, :, 0:256],
                in_=x[0:T, 0:256].rearrange("(j p) h -> p j h", p=128),
            )
            idh = cp.tile([128, 128], dt.float32r, tag="idh")
            nc.sync.dma_start(out=idh[:], in_=idp[:])
            c32 = cp.tile([128, _F32_COLS], dt.float32, tag="c32")
            # w1 pairs q=0,1 right behind so mm1 can start early
            c8 = cp.tile([128, _F8_COLS], dt.float8e4, tag="c8")
            nc.sync.dma_start(out=c8[:, 0 : 4 * PD], in_=fb8[:, 0 : 4 * PD])
            for hq in range(1, 4):
                nc.sync.dma_start(
                    out=X[0][:, :, hq * 256 : (hq + 1) * 256],
                    in_=x[0:T, hq * 256 : (hq + 1) * 256].rearrange(
                        "(j p) h -> p j h", p=128
                    ),
                )
            nc.sync.dma_start(out=c8[:, 4 * PD : _F8_COLS], in_=fb8[:, 4 * PD : _F8_COLS])
            nc.sync.dma_start(out=c32[:], in_=fb32[:])
            c16 = cp.tile([128, _F16_COLS], dt.float16, tag="c16")
            nc.sync.dma_start(out=c16[:], in_=fb16[:])
            u36v = cp.tile([KUP, H], dt.float16, tag="u36")
            nc.sync.dma_start(out=u36v[:], in_=u36p[:])
            xlast = cp.tile([128, 8], dt.float32r, tag="xlast")
            nc.sync.dma_start(
                out=xlast[:],
                in_=x.rearrange("s (c p) -> p s c", p=128)[:, S - 1, :],
            )

            w1v = c8[:, 0 : 8 * PD].rearrange("p (c n) -> p c n", c=8)
            dcv = c8[:, 8 * PD : 8 * PD + 256].rearrange("p (c n) -> p c n", c=8)
            w2v = c16[:, 0:4]
            b1v = c32[:, 0:4]
            thrv = c32[:, 4:5]
            selwv = c32[:, 5:37].rearrange("p (c a) -> p c a", c=8)
            dbv = c32[0:32, 37:38]
            selbv = c32[0:4, 38:39]
            o14v = c32[0:1, 39:43]
            i4v = c32[0:4, 43:47]
            e36v = c32[0:4, 47:83]

            # dummy gelu so the gelu table set loads during startup DMAs
            dummy = cp.tile([1, 1], dt.float32, tag="dummy")
            nc.scalar.activation(dummy[:], thrv[0:1, 0:1], AF.Gelu)

            # ---- adapter selector (once per core) ----
            ps_sel = pp.tile([4, 1], dt.float32, tag="w", bufs=2)
            for c in range(8):
                nc.tensor.matmul(
                    ps_sel[:], selwv[:, c, :], xlast[:, c : c + 1].bitcast(dt.float32),
                    start=(c == 0), stop=(c == 7),
                )
            # t = tanh((z + sel_b)/2)  -> exp(z+sel_b) = (1+t)/(1-t)
            t4 = cp.tile([4, 1], dt.float32, tag="t4")
            nc.scalar.activation(t4[:], ps_sel[:], AF.Tanh, bias=selbv, scale=0.5)
            num4 = cp.tile([4, 1], dt.float32, tag="num4")
            nc.vector.tensor_scalar(num4[:], t4[:], 1.0, None, ALU.add)
            den4 = cp.tile([4, 1], dt.float32, tag="den4")
            nc.vector.tensor_scalar(den4[:], t4[:], -1.0, 1.0, ALU.mult, ALU.add)
            rden4 = cp.tile([4, 1], dt.float32, tag="rden4")
            nc.vector.reciprocal(rden4[:], den4[:])
            e4 = cp.tile([4, 1], dt.float32, tag="e4")
            nc.vector.tensor_mul(e4[:], num4[:], rden4[:])
            ps_et = pp.tile([1, 4], dt.float32, tag="w", bufs=2)
            nc.tensor.matmul(ps_et[:], e4[:], i4v, start=True, stop=True)
            ssum = cp.tile([1, 1], dt.float32, tag="ssum")
            nc.vector.reduce_sum(ssum[:], ps_et[:], axis=mybir.AxisListType.X)
            rsum = cp.tile([1, 1], dt.float32, tag="rsum")
            nc.vector.reciprocal(rsum[:], ssum[:])
            ps_rs = pp.tile([4, 1], dt.float32, tag="w", bufs=2)
            nc.tensor.matmul(ps_rs[:], o14v, rsum[:], start=True, stop=True)
            w4 = cp.tile([4, 1], dt.float32, tag="w4")
            nc.vector.tensor_tensor(w4[:], e4[:], ps_rs[:], ALU.mult)
            ps_wv = pp.tile([KUP, 1], dt.float32, tag="w", bufs=2)
            nc.tensor.matmul(ps_wv[:], e36v, w4[:], start=True, stop=True)
            wv_sb = cp.tile([KUP, 1], dt.float32, tag="wv")
            nc.scalar.copy(wv_sb[:], ps_wv[:])
            # fold adapter weights into the up matrix, cast to fp8
            uw_sb = cp.tile([KUP, H], dt.float8e4, tag="uw")
            nc.vector.tensor_scalar(uw_sb[:], u36v[:], wv_sb[:], None, ALU.mult)

            # G buffers: gelu rows 0:32 rewritten per tile; ones rows persist
            NG = 3
            Gb = []
            for k in range(NG):
                g = cp.tile([KUP, T], dt.float8e4, tag=f"G{k}")
                nc.gpsimd.memset(g[32:KUP, :], 1.0)
                Gb.append(g)

            # ---------- per-tile emission helpers ----------
            XT = [None] * N_TILES   # each: list of 4 fp8 [128, 2, T] tiles
            Hs = [None] * N_TILES
            MK = [None] * N_TILES

            def emit_load(i):
                # one DMA per tile: fewer SP issue slots, so every load is
                # queued on the (FIFO) DMA device before the stores arrive.
                # tile 1 is H-split like tile 0 to shorten the pipeline fill.
                X[i] = wp.tile([128, 4, H], dt.float32r, tag="X", name=f"X{i}", bufs=XBUFS)
                nc.sync.dma_start(
                    out=X[i][:],
                    in_=x[i * T : (i + 1) * T, :].rearrange("(j p) h -> p j h", p=128),
                )

            def emit_transpose(i):
                # PE: 32 transposes (f32r view: 1.5 cyc/row); ACT/DVE: 8
                # psum->SBUF fp8 copies
                XT[i] = []
                for q in range(4):
                    xt8 = wp.tile([128, 2, T], dt.float8e4, tag="XT", name=f"XT{i}_{q}", bufs=8)
                    for dc in range(2):
                        c = 2 * q + dc
                        ps_xt = pp.tile([128, T], dt.float32r, tag="xt", name=f"psxt{i}_{c}", bufs=4)
                        for j in range(4):
                            nc.tensor.transpose(
                                ps_xt[:, ts(j, 128)], X[i][:, j, ts(c, 128)], idh[:]
                            )
                        if c < 6 or i >= 5:
                            # endgame tiles: keep DVE's queue clear for the
                            # STT chains; ACT takes all 8 copies
                            nc.scalar.copy(xt8[:, dc, :], ps_xt[:].bitcast(dt.float32))
                        else:
                            nc.vector.tensor_copy(xt8[:, dc, :], ps_xt[:].bitcast(dt.float32))
                    XT[i].append(xt8)

            def emit_mm1(i):
                # detector mm1 (fp8 DoubleRow) + gelu -> f16 Hs
                Hs[i] = []
                for m in range(4):
                    ps_h = pp.tile([128, T], dt.float32, tag="h", name=f"psh{i}_{m}", bufs=2)
                    for q in range(4):
                        nc.tensor.matmul(
                            ps_h[:], w1v[:, 2 * q : 2 * q + 2, ts(m, 128)], XT[i][q][:],
                            start=(q == 0), stop=(q == 3), perf_mode=DR,
                        )
                    hm = wp.tile([128, T], dt.float16, tag="Hs", name=f"Hs{i}_{m}", bufs=5)
                    nc.scalar.activation(hm[:], ps_h[:], AF.Gelu, bias=b1v[:, m : m + 1])
                    Hs[i].append(hm)

            def emit_down(i):
                # down-proj (fp8 DoubleRow) + gelu -> fp8 G rows 0:32
                ps_t = pp.tile([32, T], dt.float32, tag="w", name=f"pst{i}", bufs=2)
                for q in range(4):
                    nc.tensor.matmul(
                        ps_t[:], dcv[:, 2 * q : 2 * q + 2, :], XT[i][q][:],
                        start=(q == 0), stop=(q == 3), perf_mode=DR,
                    )
                g = Gb[i % NG]
                nc.scalar.activation(g[0:32, :], ps_t[:], AF.Gelu, bias=dbv)
                XT[i] = None

            def emit_mm2(i):
                # detector mm2, natural orientation: z[tok,1] per token chunk j
                ps_z = pp.tile([128, 4], dt.float32, tag="w", name=f"psz{i}", bufs=2)
                for j in range(4):
                    for m in range(4):
                        nc.tensor.matmul(
                            ps_z[:, j : j + 1], Hs[i][m][:, ts(j, 128)],
                            w2v[:, m : m + 1],
                            start=(m == 0), stop=(m == 3),
                        )
                maskn = wp.tile([128, 4], dt.float32, tag="maskn", name=f"maskn{i}", bufs=3)
                nc.vector.tensor_scalar(maskn[:], ps_z[:], thrv, None, ALU.is_gt)
                MK[i] = maskn
                Hs[i] = None

            def emit_up_stt(i):
                # up-proj (fp8) + fused mask*psum + residual -> f16 staging
                # tile XO; stores stream out per token chunk (HWDGE, f16)
                g = Gb[i % NG]
                last = i == N_TILES - 1
                xo = wp.tile([128, 4, H], dt.float16, tag="XO", name=f"XO{i}", bufs=6)
                for j in range(4):
                    for n in range(2):
                        ps_w = pp.tile(
                            [128, PD], dt.float32, tag="w", name=f"psw{i}_{j}_{n}", bufs=2
                        )
                        nc.tensor.matmul(
                            ps_w[:], g[:, ts(j, 128)], uw_sb[:, ts(n, PD)],
                            start=True, stop=True,
                        )
                        nc.vector.scalar_tensor_tensor(
                            xo[:, j, ts(n, PD)], ps_w[:],
                            MK[i][:, j : j + 1],
                            X[i][:, j, ts(n, PD)].bitcast(dt.float32),
                            ALU.mult, ALU.add,
                        )
                        if last:
                            # eighth-stores: the final transfer behind the
                            # last STT is only 0.125 MiB
                            nc.sync.dma_start(
                                out=out[
                                    i * T + j * 128 : i * T + (j + 1) * 128,
                                    n * PD : (n + 1) * PD,
                                ],
                                in_=xo[:, j, ts(n, PD)],
                            )
                    if (not last) and j % 2 == 1:
                        h = j // 2
                        nc.sync.dma_start(
                            out=out[
                                i * T + h * 256 : i * T + (h + 1) * 256, :
                            ].rearrange("(j p) h -> p j h", p=128),
                            in_=xo[:, 2 * h : 2 * h + 2, :],
                        )

            # ---------- software-pipelined main loop ----------
            # up+stt for tile i-1 runs in block i: its mask/G are a full
            # block old, so the DVE STT chain never waits on this block
            for i in range(1, N_TILES):
                emit_load(i)
            emit_transpose(0)
            LEAN = 2
            for i in range(N_TILES):
                emit_mm1(i)
                if 0 < i <= N_TILES - 1 - LEAN:
                    emit_up_stt(i - 1)
                emit_down(i)
                if i + 1 < N_TILES:
                    emit_transpose(i + 1)
                emit_mm2(i)
                if N_TILES - 1 - LEAN <= i < N_TILES - 1:
                    # lean tail blocks: own tile's up+stt right here so the
                    # epilogue holds only tile 7's chain
                    emit_up_stt(i)
            emit_up_stt(N_TILES - 1)

    nc.compile()
    return nc


_CACHE = {}


def _get_nc():
    if "nc" not in _CACHE:
        _CACHE["nc"] = _build()
    return _CACHE["nc"]


def _host_params(inputs):
    f32 = np.float32
    f16 = np.float16
    f8 = ml_dtypes.float8_e4m3
    pd_w1 = np.asarray(inputs["pd_w1"], f32)          # [H, PD]
    pd_b1 = np.asarray(inputs["pd_b1"], f32)          # [PD]
    pd_w2 = np.asarray(inputs["pd_w2"], f32)          # [PD, 1]
    pd_b2 = np.asarray(inputs["pd_b2"], f32)          # [1]
    down_w = np.asarray(inputs["down_w"], f32)        # [A, H, d]
    down_b = np.asarray(inputs["down_b"], f32)        # [A, d]
    up_w = np.asarray(inputs["up_w"], f32)            # [A, d, H]
    up_b = np.asarray(inputs["up_b"], f32)            # [A, H]
    sel_w = np.asarray(inputs["sel_w"], f32)          # [H, A]
    sel_b = np.asarray(inputs["sel_b"], f32)          # [A]

    # fp8 blob: w1 | dcat  (k-chunk-major per partition)
    w1s = pd_w1.reshape(8, 128, PD).transpose(1, 0, 2).reshape(128, 8 * PD)
    dcat = down_w.transpose(1, 0, 2).reshape(H, 32)
    dcats = dcat.reshape(8, 128, 32).transpose(1, 0, 2).reshape(128, 256)
    fb8 = np.concatenate([w1s, dcats], axis=1).astype(f8)
    assert fb8.shape == (128, _F8_COLS)

    # f16 blob: w2
    fb16 = pd_w2.reshape(4, 128).T.astype(f16)
    assert fb16.shape == (128, _F16_COLS)
    u36p = np.concatenate(
        [SCALE * up_w.reshape(32, H), SCALE * up_b], axis=0
    ).astype(f16)
    assert u36p.shape == (KUP, H)

    # f32 blob: b1(4) thr(1) selw(32) db(1) selb(1) o14(4) i4(4) e36(36) idh(128)
    b1s = pd_b1.reshape(4, 128).T
    thr = np.full((128, 1), math.log(THRESH / (1.0 - THRESH)) - float(pd_b2[0]), f32)
    selws = sel_w.reshape(8, 128, 4).transpose(1, 0, 2).reshape(128, 32)
    dbcol = np.zeros((128, 1), f32)
    dbcol[0:32, 0] = down_b.reshape(32)
    selbcol = np.zeros((128, 1), f32)
    selbcol[0:4, 0] = sel_b / 2.0
    o14 = np.zeros((128, 4), f32)
    o14[0, :] = 1.0
    i4m = np.zeros((128, 4), f32)
    i4m[0:4, :] = np.eye(4)
    e36m = np.zeros((128, KUP), f32)
    for r in range(32):
        e36m[r // 8, r] = 1.0
    for a in range(4):
        e36m[a, 32 + a] = 1.0
    fb32 = np.concatenate(
        [b1s, thr, selws, dbcol, selbcol, o14, i4m, e36m], axis=1
    ).astype(f32)
    assert fb32.shape == (128, _F32_COLS)

    return dict(fb8=fb8, fb16=fb16, u36p=u36p, fb32=fb32,
                idp=np.eye(128, dtype=f32))


def _run(inputs, trace=False, **kwargs):
    nc = _get_nc()
    params = _host_params(inputs)
    hs = np.asarray(inputs["hidden_states"], np.float32)
    in_maps = [dict(params, x=np.ascontiguousarray(hs[b])) for b in range(N_CORES)]
    try:
        res = run_bass_kernel_spmd(
            nc, in_maps, core_ids=list(range(N_CORES)), trace=trace, **kwargs
        )
    except ModuleNotFoundError:
        res = run_bass_kernel_spmd(
            nc, in_maps, core_ids=list(range(N_CORES)), trace=False, **kwargs
        )
    out = np.stack([res.results[b]["out"] for b in range(N_CORES)], axis=0)
    return out.astype(np.float32), res


def kernel(**inputs) -> np.ndarray:
    out, _ = _run(inputs, trace=False)
    return out
